# revision 31
# baseline (speedup 1.0000x reference)
"""DA-RNN style encoder (LSTM + input attention) on 8 Trainium2 cores.

Problem: nn_Encoder_63024350101963
  B=2048, T-1=31 steps, D=128 input feats, H=128 hidden.

Key algebraic fact exploited: in the reference,
    score = (h @ w_h + c @ w_c + b)[:, None] + x_score
the recurrent term is constant along the softmax axis, and softmax is
shift-invariant, so
    attn = softmax(x_score)      (time-constant, recurrence-independent)
Therefore weighted = attn[:,None,:] * x  is a pure elementwise op and only
the LSTM cell recurrence is serial.

v2 design notes (vs the fp32 baseline):
  * fp16 everywhere except the PSUM accumulators and the softmax
    normalization: matmuls run at 1 cycle/row (vs 4 for fp32), DVE
    elementwise gets the 2x packed mode, and the x/wt/enc DMA traffic
    halves.  Accuracy budget is rel-err < 2e-2; fp16 lands ~1e-3.
  * tanh-only transcendentals: sigmoid(z) = 0.5*tanh(z/2)+0.5, applied with
    the ACT free scale (0.5) and the affine fused into affine_mul_reduce.
    tanh and exp share one ACT table set ("exp_and_others"), so the kernel
    performs ZERO activation-table switches (the fp32 baseline paid two
    ~1.3-2.7us loads per iteration for Exp->Sigmoid).
  * W_hh matmuls take both batch subtiles in one rhs [H, 256] (N=256,
    1 cycle/row) - 4 matmuls/step instead of 8.
  * bias is a single fp16 row via a K=1 matmul (no hi/lo split needed at
    this accuracy), W_ih keeps N=512 over 2-step groups.

Device layout: feature-on-partitions, batch-on-free ("transposed") all the
way through; the host passes x pre-transposed [D, T, B_local] (fp16) and
re-transposes/upcasts the outputs, so the device never transposes the big
tensors.

PSUM layout: two ping-pong tiles [128, 2048] (4 banks each); bank c holds
gate-chunk c (pytorch order i,f,g,o; g pre-scaled 2x) for two consecutive
steps x two batch subtiles, so the bias and W_ih matmuls run at N=512 with
one weight load per two steps.  Only the W_hh matmuls (N=256) are per-step,
as the recurrence requires.

Sharding: data-parallel over batch, 8 cores x 256 rows, weights replicated.
"""

import numpy as np

T = 31          # time steps (T_ref - 1)
D = 128         # input feature dim
H = 128         # hidden dim
G = 4 * H       # gate rows
NCORES = 8
B = 2048
BL = B // NCORES  # 256 batch rows per core
BS = 128          # batch sub-tile (2 per core)
NS = BL // BS
WARMUP = 24       # PE warm-up transposes (HAM ramp), overlapped with DMA
HEADS_AHEAD = True  # emit group g+1's bias/W_ih during group g (off PE chain)
NWARM_STEP = 0    # PE keep-warm filler transposes per step (HAM stays at 2.4GHz)
NLDW_STEP = 0     # PE keep-warm standalone ldweights spins per step (hurt on HW)

_CACHE = {}


def _build_program(loop_n=0):
    from contextlib import ExitStack

    import concourse.bacc as bacc
    import concourse.mybir as mybir
    import concourse.tile as tile

    f16 = mybir.dt.float16
    f32 = mybir.dt.float32

    nc = bacc.Bacc("TRN2", target_bir_lowering=False, debug=False)

    xt_d = nc.dram_tensor("xt", [D, T, BL], f16, kind="ExternalInput").ap()
    wxb_d = nc.dram_tensor("wxb", [D, T], f32, kind="ExternalInput").ap()
    wih_d = nc.dram_tensor("wih", [D, G], f16, kind="ExternalInput").ap()
    whh_d = nc.dram_tensor("whh", [H, G], f16, kind="ExternalInput").ap()
    bias_d = nc.dram_tensor("bias", [1, G], f16, kind="ExternalInput").ap()
    ident_d = nc.dram_tensor("ident", [D, D], f16, kind="ExternalInput").ap()
    ones_d = nc.dram_tensor("ones", [1, 2 * BL], f16, kind="ExternalInput").ap()

    attn_d = nc.dram_tensor("attn_out", [D, BL], f16, kind="ExternalOutput").ap()
    enc_d = nc.dram_tensor("enc_out", [T, H, BL], f16, kind="ExternalOutput").ap()

    with ExitStack() as ctx:
        tc = ctx.enter_context(tile.TileContext(nc))

        def body():
            _emit(nc, tc, ctx, mybir, f16, f32,
                  xt_d, wxb_d, wih_d, whh_d, bias_d, ident_d, ones_d,
                  attn_d, enc_d)

        if loop_n:
            with tc.For_i(0, loop_n, 1):
                body()
        else:
            body()

    nc.compile()
    return nc


def _emit(nc, tc, ctx, mybir, f16, f32,
          xt_d, wxb_d, wih_d, whh_d, bias_d, ident_d, ones_d, attn_d, enc_d):
    from contextlib import ExitStack

    AF = mybir.ActivationFunctionType

    big = ctx.enter_context(tc.tile_pool(name="big", bufs=1))

    # ---- persistent SBUF tensors ----
    xt_s = big.tile([D, T * BL], f16, tag="xt")
    wid_s = big.tile([D, T * D], f16, tag="wid")
    wxb_s = big.tile([D, T], f32, tag="wxb")
    wxt_s = big.tile([D, T * BL], f16, tag="wxt")
    wih_s = big.tile([D, G], f16, tag="wih")
    whh_s = big.tile([H, G], f16, tag="whh")
    bias_s = big.tile([1, G], f16, tag="bias")
    ident_s = big.tile([D, D], f16, tag="ident")
    ones_s = big.tile([1, 2 * BL], f16, tag="ones")
    zro_s = big.tile([H, 2 * BS], f16, tag="zro")

    nc.sync.dma_start(out=ident_s[:], in_=ident_d[:])
    # wid = w_x[t] * I diag stack, built on the idle DVE during the front
    # (the 1MB host-built version was delaying the x chunks on the DMA path)
    nc.sync.dma_start(out=wxb_s[:], in_=wxb_d[:])
    for t in range(T):
        nc.vector.tensor_scalar_mul(
            wid_s[:, t * D:(t + 1) * D], ident_s[:], wxb_s[:, t:t + 1])
    nc.vector.memset(zro_s[:], 0.0)

    # x input chunks next (x_score consumes them as they land); the LSTM
    # weights are not needed until the recurrence starts, so they go last.
    for t0 in range(0, T, 8):
        t1 = min(t0 + 8, T)
        nc.sync.dma_start(
            out=xt_s[:, t0 * BL:t1 * BL], in_=xt_d[:, t0:t1, :])
    nc.sync.dma_start(out=wih_s[:], in_=wih_d[:])
    nc.sync.dma_start(out=whh_s[:], in_=whh_d[:])
    nc.sync.dma_start(out=bias_s[:], in_=bias_d[:])
    nc.sync.dma_start(out=ones_s[:], in_=ones_d[:])

    attnT = big.tile([D, BL], f16, tag="attnT")

    with ExitStack() as fctx:
        frs = fctx.enter_context(tc.tile_pool(name="fsmall", bufs=2))
        psf = fctx.enter_context(tc.tile_pool(name="psf", bufs=1, space="PSUM"))
        pst = fctx.enter_context(tc.tile_pool(name="pstr", bufs=2, space="PSUM"))

        # ---- PE warmup: transpose spins on ident until real work lands ----
        pwm = pst.tile([D, D], f16, tag="warm")
        for w in range(WARMUP):
            nc.tensor.transpose(pwm[:], ident_s[:], ident_s[:])

        # ---- x_score in natural [b, d]: ps_xs[j] += (xT_t chunk).T @ wid_t
        # (lhsT = x chunk so the product transposes x back; accumulate over t)
        ps_xs = [psf.tile([BS, D], f32, tag=f"xs{j}", name=f"ps_xs{j}")
                 for j in range(NS)]
        for t in range(T):
            for j in range(NS):
                nc.tensor.matmul(
                    ps_xs[j][:],
                    lhsT=xt_s[:, t * BL + j * BS: t * BL + (j + 1) * BS],
                    rhs=wid_s[:, t * D:(t + 1) * D],
                    start=(t == 0),
                    stop=(t == T - 1),
                )

        # ---- softmax straight off PSUM; transpose attn -> attnT ----
        for j in range(NS):
            nmx = frs.tile([BS, 1], f32, tag="nmx")
            nc.vector.tensor_reduce(
                nmx[:], ps_xs[j][:], axis=mybir.AxisListType.X,
                op=mybir.AluOpType.max, negate=True,
            )
            ex = frs.tile([BS, D], f32, tag="ex")
            sums = frs.tile([BS, 1], f32, tag="sums")
            nc.scalar.activation(ex[:], ps_xs[j][:], AF.Exp,
                                 bias=nmx[:], accum_out=sums[:])
            rc = frs.tile([BS, 1], f32, tag="rc")
            nc.vector.reciprocal(rc[:], sums[:])
            at = frs.tile([BS, D], f16, tag="at")
            nc.vector.tensor_scalar_mul(at[:], ex[:], rc[:])

            ptr2 = pst.tile([D, BS], f16, tag="ptr")
            nc.tensor.transpose(ptr2[:], at[:], ident_s[:])
            nc.vector.tensor_copy(attnT[:, j * BS:(j + 1) * BS], ptr2[:])

    # weighted = attn * x is separable: ship the 64KB attention vector and
    # let the host materialize the 32MB weighted output (wt_out DMA was 2MB
    # per core per iteration plus four dma_start fixed costs).
    nc.sync.dma_start(out=attn_d[:], in_=attnT[:])

    # ---- LSTM recurrence ----
    # PSUM ping-pong tiles [128, 2048]: bank c = gate chunk c (pytorch order
    # i,f,g,o; g pre-scaled 2x), holding [step tg (256 cols) | step tg+1].
    # All transcendentals are tanh: sigmoid(z) = 0.5*tanh(z/2)+0.5 with the
    # 0.5/0.5 affine fused into affine_mul_reduce; tanh shares the ACT table
    # set with the front's exp, so no table reloads ever happen.
    psg = ctx.enter_context(tc.tile_pool(name="psg", bufs=2, space="PSUM"))
    sgp = ctx.enter_context(tc.tile_pool(name="sg", bufs=6))
    sm = ctx.enter_context(tc.tile_pool(name="small", bufs=8))
    hst = ctx.enter_context(tc.tile_pool(name="hstage", bufs=3))

    mul = mybir.AluOpType.mult
    add = mybir.AluOpType.add

    # State scaling: the device carries CC = 2c and HH = 2h (W_hh pre-halved
    # on host; enc_out re-halved on host).  With sg = tanh(z/2) this makes
    # every pointwise op a single standard scalar_tensor_tensor (2x DVE mode):
    #   2*sigmoid(f)*c = 0.5*(tf+1)*CC,  2*sigmoid(i)*tanh(g) = (ti+1)*tg
    #   CC' = 0.5*A + B;  tanh(c) = tanh(0.5*CC');  HH = (to+1)*tanh(c)
    c_prev = [_Slice(zro_s[:, 0:BS]), _Slice(zro_s[:, BS:2 * BS])]
    h_prev = [_Slice(zro_s[:, 0:BS]), _Slice(zro_s[:, BS:2 * BS])]
    h_prev_full = zro_s
    hstage = None
    ngroups = (T + 1) // 2

    def emit_head(g):
        """Group head: weighted input (GPSIMD, off the DVE chain), wt_out
        flush, per-subtile PSUM tile allocs, bias + W_ih matmuls.

        Each subtile gets its OWN 2-bank PSUM tile [128, 4*256] (chunk c at
        cols c*256, two steps of 128 within), so the bank-overlap tracker
        never serializes subtile s0's ACT reads against s1's PE writes -
        that coupling was adding a full extra subtile chain to every step."""
        tg = 2 * g
        gw = min(2, T - tg)
        for t in range(tg, tg + gw):
            nc.gpsimd.tensor_mul(
                wxt_s[:, t * BL:(t + 1) * BL],
                xt_s[:, t * BL:(t + 1) * BL],
                attnT[:],
            )
        nw = gw * BL
        ps = psg.tile([128, 4 * 512], f32, tag="gates", name=f"ps_{g}")
        for c in range(4):
            gseg = slice(c * H, (c + 1) * H)
            nc.tensor.matmul(
                ps[:, c * 512:c * 512 + nw], lhsT=bias_s[0:1, gseg],
                rhs=ones_s[0:1, 0:nw], start=True, stop=False,
                skip_group_check=True,
            )
            nc.tensor.matmul(
                ps[:, c * 512:c * 512 + nw], lhsT=wih_s[:, gseg],
                rhs=wxt_s[:, tg * BL:tg * BL + nw], start=False, stop=False,
                skip_group_check=True,
            )
        return ps

    # keep-warm filler: write-only transposes on a scratch PSUM bank keep the
    # PE HAM at 2.4GHz through the per-step idle gaps (cold matmuls cost 2x).
    pwl = ctx.enter_context(tc.tile_pool(name="pwloop", bufs=1, space="PSUM"))
    pwm_l = None
    if NWARM_STEP:
        pwm_l = pwl.tile([D, D], f16, tag="warml", name="pwm_l")

    ps_next = emit_head(0)
    for g in range(ngroups):
        tg = 2 * g
        gw = min(2, T - tg)
        ps = ps_next
        if not HEADS_AHEAD and g + 1 < ngroups:
            ps_next = emit_head(g + 1)
        for dtw in range(gw):
            t = tg + dtw
            if t % 8 == 0:
                hstage = hst.tile([H, 8 * BL], f16, tag="hst")
            # W_hh for both subtiles in one rhs [H, 256] per chunk (N=256):
            # one weight load per chunk - LDWEIGHTS (~107ns each, unmodeled
            # by the cost model) dominates the real-HW chain otherwise.
            for c in range(4):
                nc.tensor.matmul(
                    ps[:, c * 512 + dtw * BL: c * 512 + (dtw + 1) * BL],
                    lhsT=whh_s[:, c * H:(c + 1) * H],
                    rhs=h_prev_full[:],
                    start=False, stop=(dtw == gw - 1),
                    skip_group_check=True,
                )
            # Emit the NEXT group's head right after this group's first
            # W_hh burst: PE is otherwise idle during the ACT/DVE chain, and
            # this keeps those 16 matmuls OFF the critical h -> W_hh path.
            if HEADS_AHEAD and dtw == 0 and g + 1 < ngroups:
                ps_next = emit_head(g + 1)
            for w in range(NWARM_STEP):
                nc.tensor.transpose(pwm_l[:], ident_s[:], ident_s[:])
            # Standalone fp16 LDWEIGHTS spins: pure PE-array activity with no
            # PSUM writes and no dependencies.  They run in the per-step idle
            # gap right after the W_hh burst and keep the HAM clock-gate at
            # 2.4GHz (a cold PE runs every matmul at half rate).  The next
            # real matmul reloads its own weights, so the clobber is safe.
            for w in range(NLDW_STEP):
                nc.tensor.ldweights(ident_s[:])
            # per-subtile tanh of the gates, then pointwise.  Emission order
            # is engine-interleaved so neither subtile's in-order engine
            # stream ever stalls behind the other's cross-engine wait.
            sgs = []
            for s in range(NS):
                slot = dtw * 2 + s
                sg = sgp.tile([128, 4 * BS], f16, tag="sg", name=f"sg_{t}_{s}")
                ps_slot = ps[:].rearrange("p (c x) -> p c x", c=4)[
                    :, :, slot * BS:(slot + 1) * BS]
                # sg = tanh(z/2): sigmoid(z) = 0.5*sg+0.5; g-rows pre-scaled
                # 2x on host so chunk 2 yields tanh(g) directly.
                nc.scalar.activation(sg[:], ps_slot, AF.Tanh, scale=0.5)
                sgs.append(sg)
            cs, tcs = [], []
            for s in range(NS):  # DVE c-chain, subtile-interleaved
                sg = sgs[s]
                ti_ = sg[:, 0 * BS:1 * BS]
                tf_ = sg[:, 1 * BS:2 * BS]
                tg_ = sg[:, 2 * BS:3 * BS]

                bv = sm.tile([H, BS], f16, tag="bv", name=f"bv_{t}_{s}")
                # B = (ti+1)*tg = 2*sigmoid(i)*tanh(g)
                # (keep on DVE: gpsimd.scalar_tensor_tensor fails at HW run)
                nc.vector.scalar_tensor_tensor(
                    out=bv[:], in0=ti_, scalar=1.0, in1=tg_, op0=add, op1=mul)
                av = sm.tile([H, BS], f16, tag="av", name=f"av_{t}_{s}")
                # A = (tf+1)*CC = 4*sigmoid(f)*c
                nc.vector.scalar_tensor_tensor(
                    out=av[:], in0=tf_, scalar=1.0, in1=c_prev[s][:],
                    op0=add, op1=mul)
                c_new = sm.tile([H, BS], f16, tag="c", name=f"c_{t}_{s}")
                # CC' = 0.5*A + B
                nc.vector.scalar_tensor_tensor(
                    out=c_new[:], in0=av[:], scalar=0.5, in1=bv[:],
                    op0=mul, op1=add)
                cs.append(c_new)
            for s in range(NS):  # ACT tanh(c)
                tc_ = sm.tile([H, BS], f16, tag="tc", name=f"tc_{t}_{s}")
                nc.scalar.activation(tc_[:], cs[s][:], AF.Tanh, scale=0.5)
                tcs.append(tc_)
            for s in range(NS):  # DVE h
                to_ = sgs[s][:, 3 * BS:4 * BS]
                h_new = hstage[:, (t % 8) * BL + s * BS:
                               (t % 8) * BL + (s + 1) * BS]
                # HH = (to+1)*tanh(c) = 2*sigmoid(o)*tanh(c)
                nc.vector.scalar_tensor_tensor(
                    out=h_new, in0=to_, scalar=1.0, in1=tcs[s][:],
                    op0=add, op1=mul)
                c_prev[s] = cs[s]
                h_prev[s] = _Slice(h_new)
            h_prev_full = _Slice(hstage[:, (t % 8) * BL:(t % 8 + 1) * BL])
            if t % 8 == 7 or t == T - 1:
                t0 = (t // 8) * 8
                n = t - t0 + 1
                nc.sync.dma_start(
                    out=enc_d[t0:t0 + n].rearrange("t h b -> h t b"),
                    in_=hstage[:].rearrange("h (t b) -> h t b", t=8)[:, :n, :],
                )


class _Slice:
    """Tiny adapter so h_prev_full[:] works for both tiles and AP slices."""

    def __init__(self, ap):
        self._ap = ap

    def __getitem__(self, key):
        return self._ap


def _get_program():
    if "nc" not in _CACHE:
        _CACHE["nc"] = _build_program()
    return _CACHE["nc"]


def _host_inputs(input_data, W_ih, W_hh, b_ih, b_hh, attn_w, attn_b):
    """Build the per-core input maps (host-side prep is weights-only +
    layout/dtype transforms)."""
    x = np.asarray(input_data, dtype=np.float32)
    W_ih = np.asarray(W_ih, dtype=np.float32)
    W_hh = np.asarray(W_hh, dtype=np.float32)
    b = (np.asarray(b_ih, dtype=np.float32)
         + np.asarray(b_hh, dtype=np.float32))
    w_x = np.asarray(attn_w, dtype=np.float32)[2 * H:]  # only the x-series part

    # scale the g-gate block (pytorch order i,f,g,o -> rows 2H:3H) by 2 so
    # tanh(z_g/2) = tanh(g) works with the single scale=0.5 tanh pass.
    scale = np.ones((G, 1), np.float32)
    scale[2 * H:3 * H] = 2.0
    wih_t = np.ascontiguousarray((W_ih * scale).T).astype(np.float16)  # [D, 4H]
    # device h-state is HH = 2h, so fold the 1/2 into W_hh
    whh_t = np.ascontiguousarray(
        (W_hh * scale).T * 0.5).astype(np.float16)                     # [H, 4H]
    bias_m = (b[None, :] * scale.T).astype(np.float16)                 # [1, 4H]

    wxb = np.ascontiguousarray(np.tile(w_x[None, :], (D, 1))).astype(np.float32)
    ident = np.eye(D, dtype=np.float16)
    ones = np.ones((1, 2 * BL), np.float16)

    in_maps = []
    for i in range(NCORES):
        xs = x[i * BL:(i + 1) * BL]                  # [BL, T, D]
        xt = np.ascontiguousarray(
            xs.transpose(2, 1, 0)).astype(np.float16)  # [D, T, BL]
        in_maps.append({
            "xt": xt,
            "wxb": wxb,
            "wih": wih_t,
            "whh": whh_t,
            "bias": bias_m,
            "ident": ident,
            "ones": ones,
        })
    return in_maps


def _gather(results, x):
    weighted = np.empty((B, T, D), np.float32)
    encoded = np.empty((B, T, H), np.float32)
    for i, r in enumerate(results):
        # weighted = attn (time-constant) * x, materialized host-side
        attn = r["attn_out"].T.astype(np.float32)          # [BL, D]
        weighted[i * BL:(i + 1) * BL] = (
            attn[:, None, :] * x[i * BL:(i + 1) * BL])
        # enc_out is [T, H, BL] fp16 and carries HH = 2h
        encoded[i * BL:(i + 1) * BL] = (
            r["enc_out"].transpose(2, 0, 1).astype(np.float32) * 0.5)
    return weighted, encoded


def kernel(input_data, W_ih, W_hh, b_ih, b_hh, attn_w, attn_b):
    from concourse.bass_utils import run_bass_kernel_spmd

    nc = _get_program()
    in_maps = _host_inputs(input_data, W_ih, W_hh, b_ih, b_hh, attn_w, attn_b)
    res = run_bass_kernel_spmd(nc, in_maps, list(range(NCORES)))
    return _gather(res.results, np.asarray(input_data, np.float32))
